# revision 21
# baseline (speedup 1.0000x reference)
"""Bilinear grid sample on 8 Trainium2 NeuronCores.

Data-parallel over batch: each core handles 2 of the 16 batches.

The host stages the image in row-pair layout (y[h,w] = [x[h,w] |
x[h+1,w]], shape [(H-1)*W, 2C] per batch) so the full 2x2 bilinear
patch for a point is ONE contiguous 4KB window: y[hf*W+wf : +2, :]
= [TL | BL | TR | BR].  One dma_gather descriptor per point (16K
descriptors/core) -- the Q7 SWDGE ucode generates descriptors at
~9ns each (measured), so descriptor count, not bytes, limits the
gather rate; row-pair staging halves it vs fetching the two rows
separately.  Window ids are int16 (max 32510 < 32767: fits).

dma_gather layout: gather position i -> partition i%128, slot
i//128.  Index tiles are [16, n/16] (position i at partition i%16,
col i//16), replicated across the 8 Q7-core partition groups.  The
host stages the raw float indices into the two layouts the device
needs (weights layout + replicated id layout) so every idx load is
one contiguous DMA; all arithmetic (floor, frac, scale, int16
conversion, corner weights) happens on-device.

Interpolation in 4-corner-weight form, all on DVE (a single compute
engine keeps every instruction at <= 1 sync wait, which this walrus
codegen requires), as a chain of scalar_tensor_tensor accumulations
against a persistent zero tile (tensor_scalar with an AP scalar
measures 1.6us/inst vs 0.47us for scalar_tensor_tensor):
  a1 = wtl*TL + 0 ; a2 = wbl*BL + a1 ; a3 = wtr*TR + a2
  och = wbr*BR + a3
A DVE "touch" of each gather tile observes the gather-DMA semaphore
once, and a post-store DVE memset observes store completion, so
cross-engine waits stay single and the tail drains' DMA waits are
redundant (_legalize_waits drops provably-implied waits via a
happens-before closure).
"""

import ml_dtypes
import numpy as np

import bass_rust
import concourse.bass as bass
import concourse.mybir as mybir
import concourse.tile as tile
from concourse import library_config
from concourse.bass_utils import run_bass_kernel_spmd
from concourse.library_overlay import lower_extended_insts

B, H, W, C, P = 16, 128, 128, 256, 8192
NCORES = 8
BPC = B // NCORES        # batches per core
GCHUNK = 1024            # points per dma_gather call (= descriptors)
CPB = P // GCHUNK        # gather chunks per batch (8)
SLOTS = GCHUNK // 128    # 128-point slots per chunk (8)
KPB = P // 128           # slots per batch (64)
NI16 = P // 16           # id columns in 16-partition layout (512)
NWIN = (H - 1) * W       # row-pair windows per batch image (16256)

_f32 = mybir.dt.float32
_i16 = mybir.dt.int16
_bf16 = mybir.dt.bfloat16
_mul = mybir.AluOpType.mult
_add = mybir.AluOpType.add
_sub = mybir.AluOpType.subtract
_Copy = mybir.ActivationFunctionType.Copy


def build_nc() -> bass.Bass:
    nc = bass.Bass("TRN2", dynamic_dma_scratch_size=32768)
    y = nc.dram_tensor("y", [BPC * NWIN, 2 * C], _bf16, kind="ExternalInput")
    # idxw[lb*128+p, 2t+c] = idx[lb, t*128+p, c]   (weights layout)
    idxw = nc.dram_tensor("idxw", [BPC * 128, 2 * KPB], _f32,
                          kind="ExternalInput")
    # idxi[lb*128+q, 2s+c] = idx[lb, s*16+(q%16), c]  (id layout, x8 repl)
    idxi = nc.dram_tensor("idxi", [BPC * 128, 2 * NI16], _f32,
                          kind="ExternalInput")
    out = nc.dram_tensor("out", [BPC * P, C], _f32, kind="ExternalOutput")

    # Overlapping-window view: window r covers y rows r and r+1
    # (1024 floats = the 2x2 patch [TL | BL | TR | BR]).
    src_win = bass_rust.AP(y[:, :].tensor, 0,
                           [[2 * C, BPC * NWIN - 1], [1, 4 * C]])

    with tile.TileContext(nc) as tc:
        with (
            tc.tile_pool(name="prep", bufs=2) as pp,
            tc.tile_pool(name="persist", bufs=1) as ps,
            tc.tile_pool(name="gp", bufs=2) as gp,
            tc.tile_pool(name="tp", bufs=2) as tp,
            tc.tile_pool(name="up", bufs=4) as up,
            tc.tile_pool(name="vp", bufs=6) as vp,
            tc.tile_pool(name="op", bufs=2) as op,
        ):
            nc.gpsimd.load_library(library_config.mlp)

            ids16 = []
            wts = []
            wtlf32 = []
            wblf32 = []
            for lb in range(BPC):
                # --- window ids, idx-stream layout (x8 replicated by host)
                rawI = pp.tile([128, 2 * NI16], _f32, tag="rawI")
                nc.sync.dma_start(rawI[:], idxi[lb * 128:(lb + 1) * 128, :])
                rndI = pp.tile([128, 2 * NI16], _f32, tag="rndI")
                nc.vector.tensor_scalar(
                    rndI[:], rawI[:], 8388608.0, 8388608.0, _add, _sub
                )
                gtI = pp.tile([128, 2 * NI16], _f32, tag="gtI")
                nc.vector.tensor_tensor(gtI[:], rndI[:], rawI[:],
                                        mybir.AluOpType.is_gt)
                flrI = pp.tile([128, 2 * NI16], _f32, tag="flrI")
                nc.vector.tensor_tensor(flrI[:], rndI[:], gtI[:], _sub)
                topf = pp.tile([128, NI16], _f32, tag="topf")
                nc.vector.scalar_tensor_tensor(
                    topf[:], flrI[:, 0::2], float(W), flrI[:, 1::2], _mul, _add
                )
                ids = ps.tile([128, NI16], _i16, tag=f"ids{lb}")
                nc.vector.tensor_scalar(
                    ids[:, :], topf[:], float(lb * NWIN), None, _add
                )
                ids16.append(ids)

            for lb in range(BPC):
                # --- corner weights (bf16), gather layout: (p,t) = t*128+p
                rawW = pp.tile([128, 2 * KPB], _f32, tag="rawW")
                nc.sync.dma_start(rawW[:], idxw[lb * 128:(lb + 1) * 128, :])
                rndW = pp.tile([128, 2 * KPB], _f32, tag="rndW")
                nc.vector.tensor_scalar(
                    rndW[:], rawW[:], 8388608.0, 8388608.0, _add, _sub
                )
                gtW = pp.tile([128, 2 * KPB], _f32, tag="gtW")
                nc.vector.tensor_tensor(gtW[:], rndW[:], rawW[:],
                                        mybir.AluOpType.is_gt)
                flrW = pp.tile([128, 2 * KPB], _f32, tag="flrW")
                nc.vector.tensor_tensor(flrW[:], rndW[:], gtW[:], _sub)
                mu = pp.tile([128, 2 * KPB], _f32, tag="mu")
                nc.vector.tensor_tensor(mu[:], rawW[:], flrW[:], _sub)
                mx = mu[:, 0::2]       # frac along h
                my = mu[:, 1::2]       # frac along w
                # corner weights: TL=(hf,wf) TR=(hf,wc) BL=(hc,wf) BR=(hc,wc)
                wbrf = pp.tile([128, KPB], _f32, tag="wbrf")
                nc.vector.tensor_tensor(wbrf[:], mx, my, _mul)
                wblf = pp.tile([128, KPB], _f32, tag="wblf")
                nc.vector.tensor_tensor(wblf[:], mx, wbrf[:], _sub)
                wtrf = pp.tile([128, KPB], _f32, tag="wtrf")
                nc.vector.tensor_tensor(wtrf[:], my, wbrf[:], _sub)
                sxy = pp.tile([128, KPB], _f32, tag="sxy")
                nc.vector.tensor_tensor(sxy[:], mx, my, _add)
                ap1 = pp.tile([128, KPB], _f32, tag="ap1")
                nc.vector.tensor_scalar(ap1[:], wbrf[:], 1.0, None, _add)
                wtlf = pp.tile([128, KPB], _f32, tag="wtlf")
                nc.vector.tensor_tensor(wtlf[:], ap1[:], sxy[:], _sub)
                ws = []
                for nm, wf in (("wtl", wtlf), ("wtr", wtrf),
                               ("wbl", wblf), ("wbr", wbrf)):
                    w16 = ps.tile([128, KPB], _bf16, tag=f"{nm}{lb}")
                    nc.vector.tensor_copy(w16[:], wf[:])
                    ws.append(w16)
                wts.append(tuple(ws))
                for nm, wf in (("wtl", wtlf), ("wbl", wblf)):
                    wp32 = ps.tile([128, KPB], _f32, tag=f"{nm}f32_{lb}")
                    nc.vector.tensor_copy(wp32[:], wf[:])
                    (wtlf32 if nm == "wtl" else wblf32).append(wp32)

            zero = ps.tile([128, C], _bf16, tag="zero")
            nc.vector.memset(zero[:], 0.0)

            # --- gather + interpolate + store
            pending = []          # och tiles whose store sem is unobserved
            for lb in range(BPC):
                ids = ids16[lb]
                wtl, wtr, wbl, wbr = wts[lb]
                for k in range(CPB):
                    i0 = k * (GCHUNK // 16)
                    i1 = (k + 1) * (GCHUNK // 16)
                    g = gp.tile([128, SLOTS, 4 * C], _bf16, tag="g")
                    nc.gpsimd.dma_gather(
                        g[:], src_win, ids[:, i0:i1],
                        GCHUNK, GCHUNK, 4 * C, elem_step=2 * C,
                    )
                    # Deferred store observation: just before this chunk
                    # rewrites the och buffer (2 chunks after its store was
                    # issued, long since drained), a memset observes the
                    # store's completion sem on DVE -- off the critical
                    # path, unlike observing right after the store issue.
                    if len(pending) >= 2:
                        nc.vector.memset(pending.pop(0)[:, :1], 0.0)
                    och = op.tile([128, SLOTS * C], _f32, tag="och")
                    # Observe the gather's DMA sem once on each consumer
                    # engine, writing into disjoint columns of the output
                    # chunk so the touches carry no other dependency.
                    nc.vector.tensor_copy(och[:, 1:2], g[:, 0, 0:1])
                    tcha = op.tile([128, 1], _f32, tag="tcha")
                    nc.scalar.activation(tcha[:], g[:, 0, 1:2], _Copy)
                    for j in range(SLOTS):
                        t = k * SLOTS + j
                        # patch blocks: [TL | BL | TR | BR]
                        # out = wtl*TL + wbl*BL + wtr*TR + wbr*BR
                        u1 = up.tile([128, C], _bf16, tag="u1")
                        nc.scalar.activation(
                            u1[:], g[:, j, 0:C], _Copy,
                            bias=0.0, scale=wtlf32[lb][:, t:t + 1],
                        )
                        u2 = up.tile([128, C], _bf16, tag="u2")
                        nc.scalar.activation(
                            u2[:], g[:, j, C:2 * C], _Copy,
                            bias=0.0, scale=wblf32[lb][:, t:t + 1],
                        )
                        v1 = vp.tile([128, C], _bf16, tag="v1")
                        nc.vector.scalar_tensor_tensor(
                            v1[:], g[:, j, 2 * C:3 * C], wtr[:, t:t + 1],
                            u1[:], _mul, _add,
                        )
                        v2 = vp.tile([128, C], _bf16, tag="v2")
                        nc.vector.scalar_tensor_tensor(
                            v2[:], g[:, j, 3 * C:4 * C], wbr[:, t:t + 1],
                            u2[:], _mul, _add,
                        )
                        nc.vector.tensor_tensor(
                            och[:, j * C:(j + 1) * C], v1[:], v2[:], _add
                        )
                    # dst[p, (j c)] = out[lb*P + (k*SLOTS+j)*128 + p, c]
                    nc.sync.dma_start(
                        bass_rust.AP(
                            out[:, :].tensor,
                            (lb * P + k * SLOTS * 128) * C,
                            [[C, 128], [128 * C, SLOTS], [1, C]],
                        ),
                        och[:],
                    )
                    pending.append(och)
            # tail: observe the final stores so the drain's DMA waits are
            # implied by DVE retirement
            for och in pending:
                nc.vector.memset(och[:, :1], 0.0)

    lower_extended_insts(nc)
    _legalize_waits(nc)
    return nc


def _legalize_waits(nc: bass.Bass) -> None:
    """Walrus codegen allows a single sync-wait per instruction.  Tile
    emits per-proc minimal waits but is not transitively minimal.  This
    pass computes a sound happens-before closure (vector clocks over
    semaphore events, walking the scheduled stream in order) and drops
    every wait already implied by the others; it asserts exactly one
    wait survives on every instruction that had several.

    Soundness notes: a proc executes its stream in order, and a wait
    stalls the proc's dispatch, so instruction i inherits all guarantees
    that held when the previous same-proc instruction dispatched.  A
    semaphore reaching value v implies the waits of the instructions
    that produced updates 1..v held; DMA-completion sems additionally
    imply the issuing instruction's engine-sem updates (completion
    happens after retirement), not vice versa."""

    def merge(a, b):
        for kk, vv in b.items():
            if a.get(kk, 0) < vv:
                a[kk] = vv

    cur: dict = {}        # proc -> VC (dict sem -> guaranteed value)
    events: dict = {}     # sem -> list of (cum_value, prefix-merged VC)
    cum: dict = {}        # sem -> cumulative update count
    # this kernel is a linear instruction stream; blocks execute in order
    for bb in nc.m.functions[0].blocks:

        def closure(s, v):
            evs = events.get(s)
            if not evs:
                return None
            for cv, vc in evs:           # events are few per sem; linear scan
                if cv >= v:
                    return vc
            return None

        for ins in bb.instructions:
            si = ins.sync_info
            eng = ins.engine
            begin = dict(cur.get(eng, {}))
            if si is not None:
                waits = list(si.on_wait)
                if len(waits) > 1:
                    # find one wait whose closure (with program-order
                    # guarantees) implies all the others
                    chosen = None
                    waits.sort(key=lambda w: w.ant_name.startswith("DMA"))
                    for w in waits:
                        trial = dict(begin)
                        c = closure(w.ant_name, w.wait_value)
                        if c is not None:
                            merge(trial, c)
                        if trial.get(w.ant_name, 0) < w.wait_value:
                            trial[w.ant_name] = w.wait_value
                        if all(trial.get(o.ant_name, 0) >= o.wait_value
                               for o in waits if o is not w):
                            chosen = w
                            begin = trial
                            break
                    assert chosen is not None, (
                        ins.name, type(ins).__name__,
                        [(w.ant_name, w.wait_value) for w in si.on_wait],
                    )
                    si.on_wait = [chosen]
                elif waits:
                    w = waits[0]
                    c = closure(w.ant_name, w.wait_value)
                    if c is not None:
                        merge(begin, c)
                    if begin.get(w.ant_name, 0) < w.wait_value:
                        begin[w.ant_name] = w.wait_value
                # register update events
                ups = list(si.on_update)
                retire = dict(begin)
                for u in ups:             # engine sems retire first
                    if not u.ant_name.startswith("DMA"):
                        cum[u.ant_name] = cum.get(u.ant_name, 0) + u.update_value
                        retire[u.ant_name] = cum[u.ant_name]
                for u in ups:
                    s = u.ant_name
                    if s.startswith("DMA"):
                        cum[s] = cum.get(s, 0) + u.update_value
                    vc = dict(retire)
                    vc[s] = cum[s]
                    prev = events.setdefault(s, [])
                    if prev:
                        base = dict(prev[-1][1])
                        merge(base, vc)
                        vc = base
                    prev.append((cum[s], vc))
            cur[eng] = begin


_NC = None


def _get_nc() -> bass.Bass:
    global _NC
    if _NC is None:
        _NC = build_nc()
    return _NC


def _in_maps(in_tensor: np.ndarray, indices: np.ndarray):
    maps = []
    for i in range(NCORES):
        xb = np.ascontiguousarray(
            in_tensor[i * BPC:(i + 1) * BPC], dtype=np.float32
        )  # [BPC, H, W, C]
        # row-pair windows: y[lb, h, w] = [x[lb,h,w,:], x[lb,h+1,w,:]]
        yb = np.concatenate([xb[:, :-1], xb[:, 1:]], axis=-1)
        yb = yb.astype(ml_dtypes.bfloat16)
        idx = np.ascontiguousarray(
            indices[i * BPC:(i + 1) * BPC], dtype=np.float32
        )  # [BPC, P, 2]
        idxw = idx.reshape(BPC, KPB, 128, 2).transpose(0, 2, 1, 3)
        base = idx.reshape(BPC, NI16, 16, 2).transpose(0, 2, 1, 3)
        idxi = np.tile(base.reshape(BPC, 16, 2 * NI16), (1, 8, 1))
        maps.append(
            {
                "y": yb.reshape(BPC * NWIN, 2 * C),
                "idxw": np.ascontiguousarray(
                    idxw.reshape(BPC * 128, 2 * KPB)
                ),
                "idxi": np.ascontiguousarray(
                    idxi.reshape(BPC * 128, 2 * NI16)
                ),
            }
        )
    return maps


def kernel(in_tensor: np.ndarray, indices: np.ndarray) -> np.ndarray:
    nc = _get_nc()
    res = run_bass_kernel_spmd(
        nc, _in_maps(in_tensor, indices), core_ids=list(range(NCORES))
    )
    return np.concatenate(
        [res.results[i]["out"].reshape(BPC, P, C) for i in range(NCORES)], axis=0
    )


# revision 22
# speedup vs baseline: 1.2311x; 1.2311x over previous
"""Bilinear grid sample on 8 Trainium2 NeuronCores.

Data-parallel over batch: each core handles 2 of the 16 batches.

The host stages the image in row-pair layout (y[h,w] = [x[h,w] |
x[h+1,w]], shape [(H-1)*W, 2C] per batch) so the full 2x2 bilinear
patch for a point is ONE contiguous 4KB window: y[hf*W+wf : +2, :]
= [TL | BL | TR | BR].  One dma_gather descriptor per point (16K
descriptors/core) -- the Q7 SWDGE ucode generates descriptors at
~9ns each (measured), so descriptor count, not bytes, limits the
gather rate; row-pair staging halves it vs fetching the two rows
separately.  Window ids are int16 (max 32510 < 32767: fits).

dma_gather layout: gather position i -> partition i%128, slot
i//128.  Index tiles are [16, n/16] (position i at partition i%16,
col i//16), replicated across the 8 Q7-core partition groups.  The
host stages the raw float indices into the two layouts the device
needs (weights layout + replicated id layout) so every idx load is
one contiguous DMA; all arithmetic (floor, frac, scale, int16
conversion, corner weights) happens on-device.

Interpolation in 4-corner-weight form, all on DVE (a single compute
engine keeps every instruction at <= 1 sync wait, which this walrus
codegen requires), as a chain of scalar_tensor_tensor accumulations
against a persistent zero tile (tensor_scalar with an AP scalar
measures 1.6us/inst vs 0.47us for scalar_tensor_tensor):
  a1 = wtl*TL + 0 ; a2 = wbl*BL + a1 ; a3 = wtr*TR + a2
  och = wbr*BR + a3
A DVE "touch" of each gather tile observes the gather-DMA semaphore
once, and a post-store DVE memset observes store completion, so
cross-engine waits stay single and the tail drains' DMA waits are
redundant (_legalize_waits drops provably-implied waits via a
happens-before closure).
"""

import ml_dtypes
import numpy as np

import bass_rust
import concourse.bass as bass
import concourse.mybir as mybir
import concourse.tile as tile
from concourse import library_config
from concourse.bass_utils import run_bass_kernel_spmd
from concourse.library_overlay import lower_extended_insts

B, H, W, C, P = 16, 128, 128, 256, 8192
NCORES = 8
BPC = B // NCORES        # batches per core
GCHUNK = 1024            # points per dma_gather call (= descriptors)
CPB = P // GCHUNK        # gather chunks per batch (8)
SLOTS = GCHUNK // 128    # 128-point slots per chunk (8)
KPB = P // 128           # slots per batch (64)
NI16 = P // 16           # id columns in 16-partition layout (512)
NWIN = (H - 1) * W       # row-pair windows per batch image (16256)

_f32 = mybir.dt.float32
_i16 = mybir.dt.int16
_bf16 = mybir.dt.bfloat16
_mul = mybir.AluOpType.mult
_add = mybir.AluOpType.add
_sub = mybir.AluOpType.subtract
_Copy = mybir.ActivationFunctionType.Copy


def build_nc() -> bass.Bass:
    nc = bass.Bass("TRN2", dynamic_dma_scratch_size=32768)
    y = nc.dram_tensor("y", [BPC * NWIN, 2 * C], _bf16, kind="ExternalInput")
    # idxw[lb*128+p, 2t+c] = idx[lb, t*128+p, c]   (weights layout)
    idxw = nc.dram_tensor("idxw", [BPC * 128, 2 * KPB], _f32,
                          kind="ExternalInput")
    # idxi[lb*128+q, 2s+c] = idx[lb, s*16+(q%16), c]  (id layout, x8 repl)
    idxi = nc.dram_tensor("idxi", [BPC * 128, 2 * NI16], _f32,
                          kind="ExternalInput")
    out = nc.dram_tensor("out", [BPC * P, C], _f32, kind="ExternalOutput")

    # Overlapping-window view: window r covers y rows r and r+1
    # (1024 floats = the 2x2 patch [TL | BL | TR | BR]).
    src_win = bass_rust.AP(y[:, :].tensor, 0,
                           [[2 * C, BPC * NWIN - 1], [1, 4 * C]])

    with tile.TileContext(nc) as tc:
        with (
            tc.tile_pool(name="prep", bufs=2) as pp,
            tc.tile_pool(name="persist", bufs=1) as ps,
            tc.tile_pool(name="gp", bufs=3) as gp,
            tc.tile_pool(name="tp", bufs=2) as tp,
            tc.tile_pool(name="up", bufs=4) as up,
            tc.tile_pool(name="vp", bufs=6) as vp,
            tc.tile_pool(name="op", bufs=3) as op,
        ):
            nc.gpsimd.load_library(library_config.mlp)

            ids16 = []
            wts = []
            wtlf32 = []
            wblf32 = []
            for lb in range(BPC):
                # --- window ids, idx-stream layout (x8 replicated by host)
                rawI = pp.tile([128, 2 * NI16], _f32, tag="rawI")
                nc.sync.dma_start(rawI[:], idxi[lb * 128:(lb + 1) * 128, :])
                rndI = pp.tile([128, 2 * NI16], _f32, tag="rndI")
                nc.vector.tensor_scalar(
                    rndI[:], rawI[:], 8388608.0, 8388608.0, _add, _sub
                )
                gtI = pp.tile([128, 2 * NI16], _f32, tag="gtI")
                nc.vector.tensor_tensor(gtI[:], rndI[:], rawI[:],
                                        mybir.AluOpType.is_gt)
                flrI = pp.tile([128, 2 * NI16], _f32, tag="flrI")
                nc.vector.tensor_tensor(flrI[:], rndI[:], gtI[:], _sub)
                topf = pp.tile([128, NI16], _f32, tag="topf")
                nc.vector.scalar_tensor_tensor(
                    topf[:], flrI[:, 0::2], float(W), flrI[:, 1::2], _mul, _add
                )
                ids = ps.tile([128, NI16], _i16, tag=f"ids{lb}")
                nc.vector.tensor_scalar(
                    ids[:, :], topf[:], float(lb * NWIN), None, _add
                )
                ids16.append(ids)

            for lb in range(BPC):
                # --- corner weights (bf16), gather layout: (p,t) = t*128+p
                rawW = pp.tile([128, 2 * KPB], _f32, tag="rawW")
                nc.sync.dma_start(rawW[:], idxw[lb * 128:(lb + 1) * 128, :])
                rndW = pp.tile([128, 2 * KPB], _f32, tag="rndW")
                nc.vector.tensor_scalar(
                    rndW[:], rawW[:], 8388608.0, 8388608.0, _add, _sub
                )
                gtW = pp.tile([128, 2 * KPB], _f32, tag="gtW")
                nc.vector.tensor_tensor(gtW[:], rndW[:], rawW[:],
                                        mybir.AluOpType.is_gt)
                flrW = pp.tile([128, 2 * KPB], _f32, tag="flrW")
                nc.vector.tensor_tensor(flrW[:], rndW[:], gtW[:], _sub)
                mu = pp.tile([128, 2 * KPB], _f32, tag="mu")
                nc.vector.tensor_tensor(mu[:], rawW[:], flrW[:], _sub)
                mx = mu[:, 0::2]       # frac along h
                my = mu[:, 1::2]       # frac along w
                # corner weights: TL=(hf,wf) TR=(hf,wc) BL=(hc,wf) BR=(hc,wc)
                wbrf = pp.tile([128, KPB], _f32, tag="wbrf")
                nc.vector.tensor_tensor(wbrf[:], mx, my, _mul)
                wblf = pp.tile([128, KPB], _f32, tag="wblf")
                nc.vector.tensor_tensor(wblf[:], mx, wbrf[:], _sub)
                wtrf = pp.tile([128, KPB], _f32, tag="wtrf")
                nc.vector.tensor_tensor(wtrf[:], my, wbrf[:], _sub)
                sxy = pp.tile([128, KPB], _f32, tag="sxy")
                nc.vector.tensor_tensor(sxy[:], mx, my, _add)
                ap1 = pp.tile([128, KPB], _f32, tag="ap1")
                nc.vector.tensor_scalar(ap1[:], wbrf[:], 1.0, None, _add)
                wtlf = pp.tile([128, KPB], _f32, tag="wtlf")
                nc.vector.tensor_tensor(wtlf[:], ap1[:], sxy[:], _sub)
                ws = []
                for nm, wf in (("wtl", wtlf), ("wtr", wtrf),
                               ("wbl", wblf), ("wbr", wbrf)):
                    w16 = ps.tile([128, KPB], _bf16, tag=f"{nm}{lb}")
                    nc.scalar.activation(w16[:], wf[:], _Copy)
                    ws.append(w16)
                wts.append(tuple(ws))
                for nm, wf in (("wtl", wtlf), ("wbl", wblf)):
                    wp32 = ps.tile([128, KPB], _f32, tag=f"{nm}f32_{lb}")
                    nc.scalar.activation(wp32[:], wf[:], _Copy)
                    (wtlf32 if nm == "wtl" else wblf32).append(wp32)

            zero = ps.tile([128, C], _bf16, tag="zero")
            nc.vector.memset(zero[:], 0.0)

            # --- gather + interpolate + store
            pending = []          # och tiles whose store sem is unobserved
            for lb in range(BPC):
                ids = ids16[lb]
                wtl, wtr, wbl, wbr = wts[lb]
                for k in range(CPB):
                    i0 = k * (GCHUNK // 16)
                    i1 = (k + 1) * (GCHUNK // 16)
                    g = gp.tile([128, SLOTS, 4 * C], _bf16, tag="g")
                    nc.gpsimd.dma_gather(
                        g[:], src_win, ids[:, i0:i1],
                        GCHUNK, GCHUNK, 4 * C, elem_step=2 * C,
                    )
                    # Deferred store observation: just before this chunk
                    # rewrites the och buffer (2 chunks after its store was
                    # issued, long since drained), a memset observes the
                    # store's completion sem on DVE -- off the critical
                    # path, unlike observing right after the store issue.
                    if len(pending) >= 3:
                        nc.vector.memset(pending.pop(0)[:, :1], 0.0)
                    och = op.tile([128, SLOTS * C], _f32, tag="och")
                    # Observe the gather's DMA sem once on each consumer
                    # engine, writing into disjoint columns of the output
                    # chunk so the touches carry no other dependency.
                    nc.vector.tensor_copy(och[:, 1:2], g[:, 0, 0:1])
                    tcha = op.tile([128, 1], _f32, tag="tcha")
                    nc.scalar.activation(tcha[:], g[:, 0, 1:2], _Copy)
                    for j in range(SLOTS):
                        t = k * SLOTS + j
                        # patch blocks: [TL | BL | TR | BR]
                        # out = wtl*TL + wbl*BL + wtr*TR + wbr*BR
                        u1 = up.tile([128, C], _bf16, tag="u1")
                        nc.scalar.activation(
                            u1[:], g[:, j, 0:C], _Copy,
                            bias=0.0, scale=wtlf32[lb][:, t:t + 1],
                        )
                        u2 = up.tile([128, C], _bf16, tag="u2")
                        nc.scalar.activation(
                            u2[:], g[:, j, C:2 * C], _Copy,
                            bias=0.0, scale=wblf32[lb][:, t:t + 1],
                        )
                        v1 = vp.tile([128, C], _bf16, tag="v1")
                        nc.vector.scalar_tensor_tensor(
                            v1[:], g[:, j, 2 * C:3 * C], wtr[:, t:t + 1],
                            u1[:], _mul, _add,
                        )
                        v2 = vp.tile([128, C], _bf16, tag="v2")
                        nc.vector.scalar_tensor_tensor(
                            v2[:], g[:, j, 3 * C:4 * C], wbr[:, t:t + 1],
                            u2[:], _mul, _add,
                        )
                        nc.vector.tensor_tensor(
                            och[:, j * C:(j + 1) * C], v1[:], v2[:], _add
                        )
                    # dst[p, (j c)] = out[lb*P + (k*SLOTS+j)*128 + p, c]
                    nc.sync.dma_start(
                        bass_rust.AP(
                            out[:, :].tensor,
                            (lb * P + k * SLOTS * 128) * C,
                            [[C, 128], [128 * C, SLOTS], [1, C]],
                        ),
                        och[:],
                    )
                    pending.append(och)
            # tail: observe the final stores so the drain's DMA waits are
            # implied by DVE retirement
            for och in pending:
                nc.vector.memset(och[:, :1], 0.0)

    lower_extended_insts(nc)
    _legalize_waits(nc)
    return nc


def _legalize_waits(nc: bass.Bass) -> None:
    """Walrus codegen allows a single sync-wait per instruction.  Tile
    emits per-proc minimal waits but is not transitively minimal.  This
    pass computes a sound happens-before closure (vector clocks over
    semaphore events, walking the scheduled stream in order) and drops
    every wait already implied by the others; it asserts exactly one
    wait survives on every instruction that had several.

    Soundness notes: a proc executes its stream in order, and a wait
    stalls the proc's dispatch, so instruction i inherits all guarantees
    that held when the previous same-proc instruction dispatched.  A
    semaphore reaching value v implies the waits of the instructions
    that produced updates 1..v held; DMA-completion sems additionally
    imply the issuing instruction's engine-sem updates (completion
    happens after retirement), not vice versa."""

    def merge(a, b):
        for kk, vv in b.items():
            if a.get(kk, 0) < vv:
                a[kk] = vv

    cur: dict = {}        # proc -> VC (dict sem -> guaranteed value)
    events: dict = {}     # sem -> list of (cum_value, prefix-merged VC)
    cum: dict = {}        # sem -> cumulative update count
    # this kernel is a linear instruction stream; blocks execute in order
    for bb in nc.m.functions[0].blocks:

        def closure(s, v):
            evs = events.get(s)
            if not evs:
                return None
            for cv, vc in evs:           # events are few per sem; linear scan
                if cv >= v:
                    return vc
            return None

        for ins in bb.instructions:
            si = ins.sync_info
            eng = ins.engine
            begin = dict(cur.get(eng, {}))
            if si is not None:
                waits = list(si.on_wait)
                if len(waits) > 1:
                    # find one wait whose closure (with program-order
                    # guarantees) implies all the others
                    chosen = None
                    waits.sort(key=lambda w: w.ant_name.startswith("DMA"))
                    for w in waits:
                        trial = dict(begin)
                        c = closure(w.ant_name, w.wait_value)
                        if c is not None:
                            merge(trial, c)
                        if trial.get(w.ant_name, 0) < w.wait_value:
                            trial[w.ant_name] = w.wait_value
                        if all(trial.get(o.ant_name, 0) >= o.wait_value
                               for o in waits if o is not w):
                            chosen = w
                            begin = trial
                            break
                    assert chosen is not None, (
                        ins.name, type(ins).__name__,
                        [(w.ant_name, w.wait_value) for w in si.on_wait],
                    )
                    si.on_wait = [chosen]
                elif waits:
                    w = waits[0]
                    c = closure(w.ant_name, w.wait_value)
                    if c is not None:
                        merge(begin, c)
                    if begin.get(w.ant_name, 0) < w.wait_value:
                        begin[w.ant_name] = w.wait_value
                # register update events
                ups = list(si.on_update)
                retire = dict(begin)
                for u in ups:             # engine sems retire first
                    if not u.ant_name.startswith("DMA"):
                        cum[u.ant_name] = cum.get(u.ant_name, 0) + u.update_value
                        retire[u.ant_name] = cum[u.ant_name]
                for u in ups:
                    s = u.ant_name
                    if s.startswith("DMA"):
                        cum[s] = cum.get(s, 0) + u.update_value
                    vc = dict(retire)
                    vc[s] = cum[s]
                    prev = events.setdefault(s, [])
                    if prev:
                        base = dict(prev[-1][1])
                        merge(base, vc)
                        vc = base
                    prev.append((cum[s], vc))
            cur[eng] = begin


_NC = None


def _get_nc() -> bass.Bass:
    global _NC
    if _NC is None:
        _NC = build_nc()
    return _NC


def _in_maps(in_tensor: np.ndarray, indices: np.ndarray):
    maps = []
    for i in range(NCORES):
        xb = np.ascontiguousarray(
            in_tensor[i * BPC:(i + 1) * BPC], dtype=np.float32
        )  # [BPC, H, W, C]
        # row-pair windows: y[lb, h, w] = [x[lb,h,w,:], x[lb,h+1,w,:]]
        yb = np.concatenate([xb[:, :-1], xb[:, 1:]], axis=-1)
        yb = yb.astype(ml_dtypes.bfloat16)
        idx = np.ascontiguousarray(
            indices[i * BPC:(i + 1) * BPC], dtype=np.float32
        )  # [BPC, P, 2]
        idxw = idx.reshape(BPC, KPB, 128, 2).transpose(0, 2, 1, 3)
        base = idx.reshape(BPC, NI16, 16, 2).transpose(0, 2, 1, 3)
        idxi = np.tile(base.reshape(BPC, 16, 2 * NI16), (1, 8, 1))
        maps.append(
            {
                "y": yb.reshape(BPC * NWIN, 2 * C),
                "idxw": np.ascontiguousarray(
                    idxw.reshape(BPC * 128, 2 * KPB)
                ),
                "idxi": np.ascontiguousarray(
                    idxi.reshape(BPC * 128, 2 * NI16)
                ),
            }
        )
    return maps


def kernel(in_tensor: np.ndarray, indices: np.ndarray) -> np.ndarray:
    nc = _get_nc()
    res = run_bass_kernel_spmd(
        nc, _in_maps(in_tensor, indices), core_ids=list(range(NCORES))
    )
    return np.concatenate(
        [res.results[i]["out"].reshape(BPC, P, C) for i in range(NCORES)], axis=0
    )


# revision 23
# speedup vs baseline: 1.3105x; 1.0646x over previous
"""Bilinear grid sample on 8 Trainium2 NeuronCores.

Data-parallel over batch: each core handles 2 of the 16 batches.

The host stages the image in row-pair layout (y[h,w] = [x[h,w] |
x[h+1,w]], shape [(H-1)*W, 2C] per batch) so the full 2x2 bilinear
patch for a point is ONE contiguous 4KB window: y[hf*W+wf : +2, :]
= [TL | BL | TR | BR].  One dma_gather descriptor per point (16K
descriptors/core) -- the Q7 SWDGE ucode generates descriptors at
~9ns each (measured), so descriptor count, not bytes, limits the
gather rate; row-pair staging halves it vs fetching the two rows
separately.  Window ids are int16 (max 32510 < 32767: fits).

dma_gather layout: gather position i -> partition i%128, slot
i//128.  Index tiles are [16, n/16] (position i at partition i%16,
col i//16), replicated across the 8 Q7-core partition groups.  The
host stages the raw float indices into the two layouts the device
needs (weights layout + replicated id layout) so every idx load is
one contiguous DMA; all arithmetic (floor, frac, scale, int16
conversion, corner weights) happens on-device.

Interpolation in 4-corner-weight form, all on DVE (a single compute
engine keeps every instruction at <= 1 sync wait, which this walrus
codegen requires), as a chain of scalar_tensor_tensor accumulations
against a persistent zero tile (tensor_scalar with an AP scalar
measures 1.6us/inst vs 0.47us for scalar_tensor_tensor):
  a1 = wtl*TL + 0 ; a2 = wbl*BL + a1 ; a3 = wtr*TR + a2
  och = wbr*BR + a3
A DVE "touch" of each gather tile observes the gather-DMA semaphore
once, and a post-store DVE memset observes store completion, so
cross-engine waits stay single and the tail drains' DMA waits are
redundant (_legalize_waits drops provably-implied waits via a
happens-before closure).
"""

import ml_dtypes
import numpy as np

import bass_rust
import concourse.bass as bass
import concourse.mybir as mybir
import concourse.tile as tile
from concourse import library_config
from concourse.bass_utils import run_bass_kernel_spmd
from concourse.library_overlay import lower_extended_insts

B, H, W, C, P = 16, 128, 128, 256, 8192
NCORES = 8
BPC = B // NCORES        # batches per core
GCHUNK = 1024            # points per dma_gather call (= descriptors)
CPB = P // GCHUNK        # gather chunks per batch (8)
SLOTS = GCHUNK // 128    # 128-point slots per chunk (8)
KPB = P // 128           # slots per batch (64)
NI16 = P // 16           # id columns in 16-partition layout (512)
NWIN = (H - 1) * W       # row-pair windows per batch image (16256)

_f32 = mybir.dt.float32
_i16 = mybir.dt.int16
_bf16 = mybir.dt.bfloat16
_mul = mybir.AluOpType.mult
_add = mybir.AluOpType.add
_sub = mybir.AluOpType.subtract
_Copy = mybir.ActivationFunctionType.Copy


def build_nc() -> bass.Bass:
    nc = bass.Bass("TRN2", dynamic_dma_scratch_size=32768)
    y = nc.dram_tensor("y", [BPC * NWIN, 2 * C], _bf16, kind="ExternalInput")
    # idxw[lb*128+p, 2t+c] = idx[lb, t*128+p, c]   (weights layout)
    idxw = nc.dram_tensor("idxw", [BPC * 128, 2 * KPB], _f32,
                          kind="ExternalInput")
    # idxi[lb*128+q, 2s+c] = idx[lb, s*16+(q%16), c]  (id layout, x8 repl)
    idxi = nc.dram_tensor("idxi", [BPC * 128, 2 * NI16], _f32,
                          kind="ExternalInput")
    out = nc.dram_tensor("out", [BPC * P, C], _f32, kind="ExternalOutput")

    # Overlapping-window view: window r covers y rows r and r+1
    # (1024 floats = the 2x2 patch [TL | BL | TR | BR]).
    src_win = bass_rust.AP(y[:, :].tensor, 0,
                           [[2 * C, BPC * NWIN - 1], [1, 4 * C]])

    with tile.TileContext(nc) as tc:
        with (
            tc.tile_pool(name="prep", bufs=2) as pp,
            tc.tile_pool(name="persist", bufs=1) as ps,
            tc.tile_pool(name="gp", bufs=3) as gp,
            tc.tile_pool(name="tp", bufs=2) as tp,
            tc.tile_pool(name="up", bufs=4) as up,
            tc.tile_pool(name="vp", bufs=2) as vp,
            tc.tile_pool(name="op", bufs=3) as op,
        ):
            nc.gpsimd.load_library(library_config.mlp)

            ids16 = []
            wts = []
            wtlf32 = []
            wblf32 = []
            for lb in range(BPC):
                # --- window ids, idx-stream layout (x8 replicated by host)
                rawI = pp.tile([128, 2 * NI16], _f32, tag="rawI")
                nc.sync.dma_start(rawI[:], idxi[lb * 128:(lb + 1) * 128, :])
                rndI = pp.tile([128, 2 * NI16], _f32, tag="rndI")
                nc.vector.tensor_scalar(
                    rndI[:], rawI[:], 8388608.0, 8388608.0, _add, _sub
                )
                gtI = pp.tile([128, 2 * NI16], _f32, tag="gtI")
                nc.vector.tensor_tensor(gtI[:], rndI[:], rawI[:],
                                        mybir.AluOpType.is_gt)
                flrI = pp.tile([128, 2 * NI16], _f32, tag="flrI")
                nc.vector.tensor_tensor(flrI[:], rndI[:], gtI[:], _sub)
                topf = pp.tile([128, NI16], _f32, tag="topf")
                nc.vector.scalar_tensor_tensor(
                    topf[:], flrI[:, 0::2], float(W), flrI[:, 1::2], _mul, _add
                )
                ids = ps.tile([128, NI16], _i16, tag=f"ids{lb}")
                nc.vector.tensor_scalar(
                    ids[:, :], topf[:], float(lb * NWIN), None, _add
                )
                ids16.append(ids)

            for lb in range(BPC):
                # --- corner weights (bf16), gather layout: (p,t) = t*128+p
                rawW = pp.tile([128, 2 * KPB], _f32, tag="rawW")
                nc.sync.dma_start(rawW[:], idxw[lb * 128:(lb + 1) * 128, :])
                rndW = pp.tile([128, 2 * KPB], _f32, tag="rndW")
                nc.vector.tensor_scalar(
                    rndW[:], rawW[:], 8388608.0, 8388608.0, _add, _sub
                )
                gtW = pp.tile([128, 2 * KPB], _f32, tag="gtW")
                nc.vector.tensor_tensor(gtW[:], rndW[:], rawW[:],
                                        mybir.AluOpType.is_gt)
                flrW = pp.tile([128, 2 * KPB], _f32, tag="flrW")
                nc.vector.tensor_tensor(flrW[:], rndW[:], gtW[:], _sub)
                mu = pp.tile([128, 2 * KPB], _f32, tag="mu")
                nc.vector.tensor_tensor(mu[:], rawW[:], flrW[:], _sub)
                mx = mu[:, 0::2]       # frac along h
                my = mu[:, 1::2]       # frac along w
                # corner weights: TL=(hf,wf) TR=(hf,wc) BL=(hc,wf) BR=(hc,wc)
                wbrf = pp.tile([128, KPB], _f32, tag="wbrf")
                nc.vector.tensor_tensor(wbrf[:], mx, my, _mul)
                wblf = pp.tile([128, KPB], _f32, tag="wblf")
                nc.vector.tensor_tensor(wblf[:], mx, wbrf[:], _sub)
                wtrf = pp.tile([128, KPB], _f32, tag="wtrf")
                nc.vector.tensor_tensor(wtrf[:], my, wbrf[:], _sub)
                sxy = pp.tile([128, KPB], _f32, tag="sxy")
                nc.vector.tensor_tensor(sxy[:], mx, my, _add)
                ap1 = pp.tile([128, KPB], _f32, tag="ap1")
                nc.vector.tensor_scalar(ap1[:], wbrf[:], 1.0, None, _add)
                wtlf = pp.tile([128, KPB], _f32, tag="wtlf")
                nc.vector.tensor_tensor(wtlf[:], ap1[:], sxy[:], _sub)
                ws = []
                for nm, wf in (("wtl", wtlf), ("wtr", wtrf),
                               ("wbl", wblf), ("wbr", wbrf)):
                    w16 = ps.tile([128, KPB], _bf16, tag=f"{nm}{lb}")
                    nc.scalar.activation(w16[:], wf[:], _Copy)
                    ws.append(w16)
                wts.append(tuple(ws))
                for nm, wf in (("wtl", wtlf), ("wbl", wblf)):
                    wp32 = ps.tile([128, KPB], _f32, tag=f"{nm}f32_{lb}")
                    nc.scalar.activation(wp32[:], wf[:], _Copy)
                    (wtlf32 if nm == "wtl" else wblf32).append(wp32)

            zero = ps.tile([128, C], _bf16, tag="zero")
            nc.vector.memset(zero[:], 0.0)

            # --- gather + interpolate + store
            pending = []          # och tiles whose store sem is unobserved
            for lb in range(BPC):
                ids = ids16[lb]
                wtl, wtr, wbl, wbr = wts[lb]
                for k in range(CPB):
                    i0 = k * (GCHUNK // 16)
                    i1 = (k + 1) * (GCHUNK // 16)
                    g = gp.tile([128, SLOTS, 4 * C], _bf16, tag="g")
                    nc.gpsimd.dma_gather(
                        g[:], src_win, ids[:, i0:i1],
                        GCHUNK, GCHUNK, 4 * C, elem_step=2 * C,
                    )
                    # Deferred store observation: just before this chunk
                    # rewrites the och buffer (2 chunks after its store was
                    # issued, long since drained), a memset observes the
                    # store's completion sem on DVE -- off the critical
                    # path, unlike observing right after the store issue.
                    if len(pending) >= 3:
                        nc.vector.memset(pending.pop(0)[:, :1], 0.0)
                    och = op.tile([128, SLOTS * C], _f32, tag="och")
                    # Observe the gather's DMA sem once on each consumer
                    # engine, writing into disjoint columns of the output
                    # chunk so the touches carry no other dependency.
                    nc.vector.tensor_copy(och[:, 1:2], g[:, 0, 0:1])
                    tcha = op.tile([128, 1], _f32, tag="tcha")
                    nc.scalar.activation(tcha[:], g[:, 0, 1:2], _Copy)
                    vg1 = vp.tile([128, SLOTS, C], _bf16, tag="vg1")
                    vg2 = vp.tile([128, SLOTS, C], _bf16, tag="vg2")
                    for j in range(SLOTS):
                        t = k * SLOTS + j
                        # patch blocks: [TL | BL | TR | BR]
                        # out = wtl*TL + wbl*BL + wtr*TR + wbr*BR
                        u1 = up.tile([128, C], _bf16, tag="u1")
                        nc.scalar.activation(
                            u1[:], g[:, j, 0:C], _Copy,
                            bias=0.0, scale=wtlf32[lb][:, t:t + 1],
                        )
                        u2 = up.tile([128, C], _bf16, tag="u2")
                        nc.scalar.activation(
                            u2[:], g[:, j, C:2 * C], _Copy,
                            bias=0.0, scale=wblf32[lb][:, t:t + 1],
                        )
                        nc.vector.scalar_tensor_tensor(
                            vg1[:, j, :], g[:, j, 2 * C:3 * C], wtr[:, t:t + 1],
                            u1[:], _mul, _add,
                        )
                        nc.vector.scalar_tensor_tensor(
                            vg2[:, j, :], g[:, j, 3 * C:4 * C], wbr[:, t:t + 1],
                            u2[:], _mul, _add,
                        )
                    nc.vector.tensor_tensor(
                        och[:].rearrange("p (a b) -> p a b", a=SLOTS),
                        vg1[:], vg2[:], _add)
                    # dst[p, (j c)] = out[lb*P + (k*SLOTS+j)*128 + p, c]
                    nc.sync.dma_start(
                        bass_rust.AP(
                            out[:, :].tensor,
                            (lb * P + k * SLOTS * 128) * C,
                            [[C, 128], [128 * C, SLOTS], [1, C]],
                        ),
                        och[:],
                    )
                    pending.append(och)
            # tail: observe the final stores so the drain's DMA waits are
            # implied by DVE retirement
            for och in pending:
                nc.vector.memset(och[:, :1], 0.0)

    lower_extended_insts(nc)
    _legalize_waits(nc)
    return nc


def _legalize_waits(nc: bass.Bass) -> None:
    """Walrus codegen allows a single sync-wait per instruction.  Tile
    emits per-proc minimal waits but is not transitively minimal.  This
    pass computes a sound happens-before closure (vector clocks over
    semaphore events, walking the scheduled stream in order) and drops
    every wait already implied by the others; it asserts exactly one
    wait survives on every instruction that had several.

    Soundness notes: a proc executes its stream in order, and a wait
    stalls the proc's dispatch, so instruction i inherits all guarantees
    that held when the previous same-proc instruction dispatched.  A
    semaphore reaching value v implies the waits of the instructions
    that produced updates 1..v held; DMA-completion sems additionally
    imply the issuing instruction's engine-sem updates (completion
    happens after retirement), not vice versa."""

    def merge(a, b):
        for kk, vv in b.items():
            if a.get(kk, 0) < vv:
                a[kk] = vv

    cur: dict = {}        # proc -> VC (dict sem -> guaranteed value)
    events: dict = {}     # sem -> list of (cum_value, prefix-merged VC)
    cum: dict = {}        # sem -> cumulative update count
    # this kernel is a linear instruction stream; blocks execute in order
    for bb in nc.m.functions[0].blocks:

        def closure(s, v):
            evs = events.get(s)
            if not evs:
                return None
            for cv, vc in evs:           # events are few per sem; linear scan
                if cv >= v:
                    return vc
            return None

        for ins in bb.instructions:
            si = ins.sync_info
            eng = ins.engine
            begin = dict(cur.get(eng, {}))
            if si is not None:
                waits = list(si.on_wait)
                if len(waits) > 1:
                    # find one wait whose closure (with program-order
                    # guarantees) implies all the others
                    chosen = None
                    waits.sort(key=lambda w: w.ant_name.startswith("DMA"))
                    for w in waits:
                        trial = dict(begin)
                        c = closure(w.ant_name, w.wait_value)
                        if c is not None:
                            merge(trial, c)
                        if trial.get(w.ant_name, 0) < w.wait_value:
                            trial[w.ant_name] = w.wait_value
                        if all(trial.get(o.ant_name, 0) >= o.wait_value
                               for o in waits if o is not w):
                            chosen = w
                            begin = trial
                            break
                    assert chosen is not None, (
                        ins.name, type(ins).__name__,
                        [(w.ant_name, w.wait_value) for w in si.on_wait],
                    )
                    si.on_wait = [chosen]
                elif waits:
                    w = waits[0]
                    c = closure(w.ant_name, w.wait_value)
                    if c is not None:
                        merge(begin, c)
                    if begin.get(w.ant_name, 0) < w.wait_value:
                        begin[w.ant_name] = w.wait_value
                # register update events
                ups = list(si.on_update)
                retire = dict(begin)
                for u in ups:             # engine sems retire first
                    if not u.ant_name.startswith("DMA"):
                        cum[u.ant_name] = cum.get(u.ant_name, 0) + u.update_value
                        retire[u.ant_name] = cum[u.ant_name]
                for u in ups:
                    s = u.ant_name
                    if s.startswith("DMA"):
                        cum[s] = cum.get(s, 0) + u.update_value
                    vc = dict(retire)
                    vc[s] = cum[s]
                    prev = events.setdefault(s, [])
                    if prev:
                        base = dict(prev[-1][1])
                        merge(base, vc)
                        vc = base
                    prev.append((cum[s], vc))
            cur[eng] = begin


_NC = None


def _get_nc() -> bass.Bass:
    global _NC
    if _NC is None:
        _NC = build_nc()
    return _NC


def _in_maps(in_tensor: np.ndarray, indices: np.ndarray):
    maps = []
    for i in range(NCORES):
        xb = np.ascontiguousarray(
            in_tensor[i * BPC:(i + 1) * BPC], dtype=np.float32
        )  # [BPC, H, W, C]
        # row-pair windows: y[lb, h, w] = [x[lb,h,w,:], x[lb,h+1,w,:]]
        yb = np.concatenate([xb[:, :-1], xb[:, 1:]], axis=-1)
        yb = yb.astype(ml_dtypes.bfloat16)
        idx = np.ascontiguousarray(
            indices[i * BPC:(i + 1) * BPC], dtype=np.float32
        )  # [BPC, P, 2]
        idxw = idx.reshape(BPC, KPB, 128, 2).transpose(0, 2, 1, 3)
        base = idx.reshape(BPC, NI16, 16, 2).transpose(0, 2, 1, 3)
        idxi = np.tile(base.reshape(BPC, 16, 2 * NI16), (1, 8, 1))
        maps.append(
            {
                "y": yb.reshape(BPC * NWIN, 2 * C),
                "idxw": np.ascontiguousarray(
                    idxw.reshape(BPC * 128, 2 * KPB)
                ),
                "idxi": np.ascontiguousarray(
                    idxi.reshape(BPC * 128, 2 * NI16)
                ),
            }
        )
    return maps


def kernel(in_tensor: np.ndarray, indices: np.ndarray) -> np.ndarray:
    nc = _get_nc()
    res = run_bass_kernel_spmd(
        nc, _in_maps(in_tensor, indices), core_ids=list(range(NCORES))
    )
    return np.concatenate(
        [res.results[i]["out"].reshape(BPC, P, C) for i in range(NCORES)], axis=0
    )


# revision 24
# speedup vs baseline: 1.3580x; 1.0362x over previous
"""Bilinear grid sample on 8 Trainium2 NeuronCores.

Data-parallel over batch: each core handles 2 of the 16 batches.

The host stages the image in row-pair layout (y[h,w] = [x[h,w] |
x[h+1,w]], shape [(H-1)*W, 2C] per batch) so the full 2x2 bilinear
patch for a point is ONE contiguous 4KB window: y[hf*W+wf : +2, :]
= [TL | BL | TR | BR].  One dma_gather descriptor per point (16K
descriptors/core) -- the Q7 SWDGE ucode generates descriptors at
~9ns each (measured), so descriptor count, not bytes, limits the
gather rate; row-pair staging halves it vs fetching the two rows
separately.  Window ids are int16 (max 32510 < 32767: fits).

dma_gather layout: gather position i -> partition i%128, slot
i//128.  Index tiles are [16, n/16] (position i at partition i%16,
col i//16), replicated across the 8 Q7-core partition groups.  The
host stages the raw float indices into the two layouts the device
needs (weights layout + replicated id layout) so every idx load is
one contiguous DMA; all arithmetic (floor, frac, scale, int16
conversion, corner weights) happens on-device.

Interpolation in 4-corner-weight form, all on DVE (a single compute
engine keeps every instruction at <= 1 sync wait, which this walrus
codegen requires), as a chain of scalar_tensor_tensor accumulations
against a persistent zero tile (tensor_scalar with an AP scalar
measures 1.6us/inst vs 0.47us for scalar_tensor_tensor):
  a1 = wtl*TL + 0 ; a2 = wbl*BL + a1 ; a3 = wtr*TR + a2
  och = wbr*BR + a3
A DVE "touch" of each gather tile observes the gather-DMA semaphore
once, and a post-store DVE memset observes store completion, so
cross-engine waits stay single and the tail drains' DMA waits are
redundant (_legalize_waits drops provably-implied waits via a
happens-before closure).
"""

import ml_dtypes
import numpy as np

import bass_rust
import concourse.bass as bass
import concourse.mybir as mybir
import concourse.tile as tile
from concourse import library_config
from concourse.bass_utils import run_bass_kernel_spmd
from concourse.library_overlay import lower_extended_insts

B, H, W, C, P = 16, 128, 128, 256, 8192
NCORES = 8
BPC = B // NCORES        # batches per core
GCHUNK = 1024            # points per dma_gather call (= descriptors)
CPB = P // GCHUNK        # gather chunks per batch (8)
SLOTS = GCHUNK // 128    # 128-point slots per chunk (8)
KPB = P // 128           # slots per batch (64)
NI16 = P // 16           # id columns in 16-partition layout (512)
NWIN = (H - 1) * W       # row-pair windows per batch image (16256)

_f32 = mybir.dt.float32
_i16 = mybir.dt.int16
_bf16 = mybir.dt.bfloat16
_mul = mybir.AluOpType.mult
_add = mybir.AluOpType.add
_sub = mybir.AluOpType.subtract
_Copy = mybir.ActivationFunctionType.Copy


def build_nc() -> bass.Bass:
    nc = bass.Bass("TRN2", dynamic_dma_scratch_size=32768)
    y = nc.dram_tensor("y", [BPC * NWIN, 2 * C], _bf16, kind="ExternalInput")
    # idxw[lb*128+p, 2t+c] = idx[lb, t*128+p, c]   (weights layout)
    idxw = nc.dram_tensor("idxw", [BPC * 128, 2 * KPB], _f32,
                          kind="ExternalInput")
    # idxi[lb*128+q, 2s+c] = idx[lb, s*16+(q%16), c]  (id layout, x8 repl)
    idxi = nc.dram_tensor("idxi", [BPC * 128, 2 * NI16], _f32,
                          kind="ExternalInput")
    out = nc.dram_tensor("out", [BPC * P, C], _f32, kind="ExternalOutput")

    # Overlapping-window view: window r covers y rows r and r+1
    # (1024 floats = the 2x2 patch [TL | BL | TR | BR]).
    src_win = bass_rust.AP(y[:, :].tensor, 0,
                           [[2 * C, BPC * NWIN - 1], [1, 4 * C]])

    with tile.TileContext(nc) as tc:
        with (
            tc.tile_pool(name="prep", bufs=2) as pp,
            tc.tile_pool(name="persist", bufs=1) as ps,
            tc.tile_pool(name="gp", bufs=4) as gp,
            tc.tile_pool(name="tp", bufs=2) as tp,
            tc.tile_pool(name="up", bufs=6) as up,
            tc.tile_pool(name="vp", bufs=2) as vp,
            tc.tile_pool(name="op", bufs=4) as op,
        ):
            nc.gpsimd.load_library(library_config.mlp)

            ids16 = []
            wts = []
            wtlf32 = []
            wblf32 = []
            for lb in range(BPC):
                # --- window ids, idx-stream layout (x8 replicated by host)
                rawI = pp.tile([128, 2 * NI16], _f32, tag="rawI")
                nc.sync.dma_start(rawI[:], idxi[lb * 128:(lb + 1) * 128, :])
                rndI = pp.tile([128, 2 * NI16], _f32, tag="rndI")
                nc.vector.tensor_scalar(
                    rndI[:], rawI[:], 8388608.0, 8388608.0, _add, _sub
                )
                gtI = pp.tile([128, 2 * NI16], _f32, tag="gtI")
                nc.vector.tensor_tensor(gtI[:], rndI[:], rawI[:],
                                        mybir.AluOpType.is_gt)
                flrI = pp.tile([128, 2 * NI16], _f32, tag="flrI")
                nc.vector.tensor_tensor(flrI[:], rndI[:], gtI[:], _sub)
                topf = pp.tile([128, NI16], _f32, tag="topf")
                nc.vector.scalar_tensor_tensor(
                    topf[:], flrI[:, 0::2], float(W), flrI[:, 1::2], _mul, _add
                )
                ids = ps.tile([128, NI16], _i16, tag=f"ids{lb}")
                nc.vector.tensor_scalar(
                    ids[:, :], topf[:], float(lb * NWIN), None, _add
                )
                ids16.append(ids)

            for lb in range(BPC):
                # --- corner weights (bf16), gather layout: (p,t) = t*128+p
                rawW = pp.tile([128, 2 * KPB], _f32, tag="rawW")
                nc.sync.dma_start(rawW[:], idxw[lb * 128:(lb + 1) * 128, :])
                rndW = pp.tile([128, 2 * KPB], _f32, tag="rndW")
                nc.vector.tensor_scalar(
                    rndW[:], rawW[:], 8388608.0, 8388608.0, _add, _sub
                )
                gtW = pp.tile([128, 2 * KPB], _f32, tag="gtW")
                nc.vector.tensor_tensor(gtW[:], rndW[:], rawW[:],
                                        mybir.AluOpType.is_gt)
                flrW = pp.tile([128, 2 * KPB], _f32, tag="flrW")
                nc.vector.tensor_tensor(flrW[:], rndW[:], gtW[:], _sub)
                mu = pp.tile([128, 2 * KPB], _f32, tag="mu")
                nc.vector.tensor_tensor(mu[:], rawW[:], flrW[:], _sub)
                mx = mu[:, 0::2]       # frac along h
                my = mu[:, 1::2]       # frac along w
                # corner weights: TL=(hf,wf) TR=(hf,wc) BL=(hc,wf) BR=(hc,wc)
                wbrf = pp.tile([128, KPB], _f32, tag="wbrf")
                nc.vector.tensor_tensor(wbrf[:], mx, my, _mul)
                wblf = pp.tile([128, KPB], _f32, tag="wblf")
                nc.vector.tensor_tensor(wblf[:], mx, wbrf[:], _sub)
                wtrf = pp.tile([128, KPB], _f32, tag="wtrf")
                nc.vector.tensor_tensor(wtrf[:], my, wbrf[:], _sub)
                sxy = pp.tile([128, KPB], _f32, tag="sxy")
                nc.vector.tensor_tensor(sxy[:], mx, my, _add)
                ap1 = pp.tile([128, KPB], _f32, tag="ap1")
                nc.vector.tensor_scalar(ap1[:], wbrf[:], 1.0, None, _add)
                wtlf = pp.tile([128, KPB], _f32, tag="wtlf")
                nc.vector.tensor_tensor(wtlf[:], ap1[:], sxy[:], _sub)
                ws = []
                for nm, wf in (("wtl", wtlf), ("wtr", wtrf),
                               ("wbl", wblf), ("wbr", wbrf)):
                    w16 = ps.tile([128, KPB], _bf16, tag=f"{nm}{lb}")
                    nc.scalar.activation(w16[:], wf[:], _Copy)
                    ws.append(w16)
                wts.append(tuple(ws))
                for nm, wf in (("wtl", wtlf), ("wbl", wblf)):
                    wp32 = ps.tile([128, KPB], _f32, tag=f"{nm}f32_{lb}")
                    nc.scalar.activation(wp32[:], wf[:], _Copy)
                    (wtlf32 if nm == "wtl" else wblf32).append(wp32)

            zero = ps.tile([128, C], _bf16, tag="zero")
            nc.vector.memset(zero[:], 0.0)

            # --- gather + interpolate + store
            pending = []          # och tiles whose store sem is unobserved
            for lb in range(BPC):
                ids = ids16[lb]
                wtl, wtr, wbl, wbr = wts[lb]
                for k in range(CPB):
                    i0 = k * (GCHUNK // 16)
                    i1 = (k + 1) * (GCHUNK // 16)
                    g = gp.tile([128, SLOTS, 4 * C], _bf16, tag="g")
                    nc.gpsimd.dma_gather(
                        g[:], src_win, ids[:, i0:i1],
                        GCHUNK, GCHUNK, 4 * C, elem_step=2 * C,
                    )
                    # Deferred store observation: just before this chunk
                    # rewrites the och buffer (2 chunks after its store was
                    # issued, long since drained), a memset observes the
                    # store's completion sem on DVE -- off the critical
                    # path, unlike observing right after the store issue.
                    if len(pending) >= 4:
                        nc.vector.memset(pending.pop(0)[:, :1], 0.0)
                    och = op.tile([128, SLOTS * C], _f32, tag="och")
                    # Observe the gather's DMA sem once on each consumer
                    # engine, writing into disjoint columns of the output
                    # chunk so the touches carry no other dependency.
                    nc.vector.tensor_copy(och[:, 1:2], g[:, 0, 0:1])
                    tcha = op.tile([128, 1], _f32, tag="tcha")
                    nc.scalar.activation(tcha[:], g[:, 0, 1:2], _Copy)
                    vg1 = vp.tile([128, SLOTS, C], _bf16, tag="vg1")
                    vg2 = vp.tile([128, SLOTS, C], _bf16, tag="vg2")
                    for j in range(SLOTS):
                        t = k * SLOTS + j
                        # patch blocks: [TL | BL | TR | BR]
                        # out = wtl*TL + wbl*BL + wtr*TR + wbr*BR
                        u1 = up.tile([128, C], _bf16, tag="u1")
                        nc.scalar.activation(
                            u1[:], g[:, j, 0:C], _Copy,
                            bias=0.0, scale=wtlf32[lb][:, t:t + 1],
                        )
                        u2 = up.tile([128, C], _bf16, tag="u2")
                        nc.scalar.activation(
                            u2[:], g[:, j, C:2 * C], _Copy,
                            bias=0.0, scale=wblf32[lb][:, t:t + 1],
                        )
                        nc.vector.scalar_tensor_tensor(
                            vg1[:, j, :], g[:, j, 2 * C:3 * C], wtr[:, t:t + 1],
                            u1[:], _mul, _add,
                        )
                        nc.vector.scalar_tensor_tensor(
                            vg2[:, j, :], g[:, j, 3 * C:4 * C], wbr[:, t:t + 1],
                            u2[:], _mul, _add,
                        )
                    nc.vector.tensor_tensor(
                        och[:].rearrange("p (a b) -> p a b", a=SLOTS),
                        vg1[:], vg2[:], _add)
                    # dst[p, (j c)] = out[lb*P + (k*SLOTS+j)*128 + p, c]
                    nc.sync.dma_start(
                        bass_rust.AP(
                            out[:, :].tensor,
                            (lb * P + k * SLOTS * 128) * C,
                            [[C, 128], [128 * C, SLOTS], [1, C]],
                        ),
                        och[:],
                    )
                    pending.append(och)
            # tail: observe the final stores so the drain's DMA waits are
            # implied by DVE retirement
            for och in pending:
                nc.vector.memset(och[:, :1], 0.0)

    lower_extended_insts(nc)
    _legalize_waits(nc)
    return nc


def _legalize_waits(nc: bass.Bass) -> None:
    """Walrus codegen allows a single sync-wait per instruction.  Tile
    emits per-proc minimal waits but is not transitively minimal.  This
    pass computes a sound happens-before closure (vector clocks over
    semaphore events, walking the scheduled stream in order) and drops
    every wait already implied by the others; it asserts exactly one
    wait survives on every instruction that had several.

    Soundness notes: a proc executes its stream in order, and a wait
    stalls the proc's dispatch, so instruction i inherits all guarantees
    that held when the previous same-proc instruction dispatched.  A
    semaphore reaching value v implies the waits of the instructions
    that produced updates 1..v held; DMA-completion sems additionally
    imply the issuing instruction's engine-sem updates (completion
    happens after retirement), not vice versa."""

    def merge(a, b):
        for kk, vv in b.items():
            if a.get(kk, 0) < vv:
                a[kk] = vv

    cur: dict = {}        # proc -> VC (dict sem -> guaranteed value)
    events: dict = {}     # sem -> list of (cum_value, prefix-merged VC)
    cum: dict = {}        # sem -> cumulative update count
    # this kernel is a linear instruction stream; blocks execute in order
    for bb in nc.m.functions[0].blocks:

        def closure(s, v):
            evs = events.get(s)
            if not evs:
                return None
            for cv, vc in evs:           # events are few per sem; linear scan
                if cv >= v:
                    return vc
            return None

        for ins in bb.instructions:
            si = ins.sync_info
            eng = ins.engine
            begin = dict(cur.get(eng, {}))
            if si is not None:
                waits = list(si.on_wait)
                if len(waits) > 1:
                    # find one wait whose closure (with program-order
                    # guarantees) implies all the others
                    chosen = None
                    waits.sort(key=lambda w: w.ant_name.startswith("DMA"))
                    for w in waits:
                        trial = dict(begin)
                        c = closure(w.ant_name, w.wait_value)
                        if c is not None:
                            merge(trial, c)
                        if trial.get(w.ant_name, 0) < w.wait_value:
                            trial[w.ant_name] = w.wait_value
                        if all(trial.get(o.ant_name, 0) >= o.wait_value
                               for o in waits if o is not w):
                            chosen = w
                            begin = trial
                            break
                    assert chosen is not None, (
                        ins.name, type(ins).__name__,
                        [(w.ant_name, w.wait_value) for w in si.on_wait],
                    )
                    si.on_wait = [chosen]
                elif waits:
                    w = waits[0]
                    c = closure(w.ant_name, w.wait_value)
                    if c is not None:
                        merge(begin, c)
                    if begin.get(w.ant_name, 0) < w.wait_value:
                        begin[w.ant_name] = w.wait_value
                # register update events
                ups = list(si.on_update)
                retire = dict(begin)
                for u in ups:             # engine sems retire first
                    if not u.ant_name.startswith("DMA"):
                        cum[u.ant_name] = cum.get(u.ant_name, 0) + u.update_value
                        retire[u.ant_name] = cum[u.ant_name]
                for u in ups:
                    s = u.ant_name
                    if s.startswith("DMA"):
                        cum[s] = cum.get(s, 0) + u.update_value
                    vc = dict(retire)
                    vc[s] = cum[s]
                    prev = events.setdefault(s, [])
                    if prev:
                        base = dict(prev[-1][1])
                        merge(base, vc)
                        vc = base
                    prev.append((cum[s], vc))
            cur[eng] = begin


_NC = None


def _get_nc() -> bass.Bass:
    global _NC
    if _NC is None:
        _NC = build_nc()
    return _NC


def _in_maps(in_tensor: np.ndarray, indices: np.ndarray):
    maps = []
    for i in range(NCORES):
        xb = np.ascontiguousarray(
            in_tensor[i * BPC:(i + 1) * BPC], dtype=np.float32
        )  # [BPC, H, W, C]
        # row-pair windows: y[lb, h, w] = [x[lb,h,w,:], x[lb,h+1,w,:]]
        yb = np.concatenate([xb[:, :-1], xb[:, 1:]], axis=-1)
        yb = yb.astype(ml_dtypes.bfloat16)
        idx = np.ascontiguousarray(
            indices[i * BPC:(i + 1) * BPC], dtype=np.float32
        )  # [BPC, P, 2]
        idxw = idx.reshape(BPC, KPB, 128, 2).transpose(0, 2, 1, 3)
        base = idx.reshape(BPC, NI16, 16, 2).transpose(0, 2, 1, 3)
        idxi = np.tile(base.reshape(BPC, 16, 2 * NI16), (1, 8, 1))
        maps.append(
            {
                "y": yb.reshape(BPC * NWIN, 2 * C),
                "idxw": np.ascontiguousarray(
                    idxw.reshape(BPC * 128, 2 * KPB)
                ),
                "idxi": np.ascontiguousarray(
                    idxi.reshape(BPC * 128, 2 * NI16)
                ),
            }
        )
    return maps


def kernel(in_tensor: np.ndarray, indices: np.ndarray) -> np.ndarray:
    nc = _get_nc()
    res = run_bass_kernel_spmd(
        nc, _in_maps(in_tensor, indices), core_ids=list(range(NCORES))
    )
    return np.concatenate(
        [res.results[i]["out"].reshape(BPC, P, C) for i in range(NCORES)], axis=0
    )


# revision 29
# speedup vs baseline: 1.3763x; 1.0135x over previous
"""Bilinear grid sample on 8 Trainium2 NeuronCores.

Data-parallel over batch: each core handles 2 of the 16 batches.

The host stages the image in row-pair layout (y[h,w] = [x[h,w] |
x[h+1,w]], shape [(H-1)*W, 2C] per batch) so the full 2x2 bilinear
patch for a point is ONE contiguous 4KB window: y[hf*W+wf : +2, :]
= [TL | BL | TR | BR].  One dma_gather descriptor per point (16K
descriptors/core) -- the Q7 SWDGE ucode generates descriptors at
~9ns each (measured), so descriptor count, not bytes, limits the
gather rate; row-pair staging halves it vs fetching the two rows
separately.  Window ids are int16 (max 32510 < 32767: fits).

dma_gather layout: gather position i -> partition i%128, slot
i//128.  Index tiles are [16, n/16] (position i at partition i%16,
col i//16), replicated across the 8 Q7-core partition groups.  The
host stages the raw float indices into the two layouts the device
needs (weights layout + replicated id layout) so every idx load is
one contiguous DMA; all arithmetic (floor, frac, scale, int16
conversion, corner weights) happens on-device.

Interpolation in 4-corner-weight form, all on DVE (a single compute
engine keeps every instruction at <= 1 sync wait, which this walrus
codegen requires), as a chain of scalar_tensor_tensor accumulations
against a persistent zero tile (tensor_scalar with an AP scalar
measures 1.6us/inst vs 0.47us for scalar_tensor_tensor):
  a1 = wtl*TL + 0 ; a2 = wbl*BL + a1 ; a3 = wtr*TR + a2
  och = wbr*BR + a3
A DVE "touch" of each gather tile observes the gather-DMA semaphore
once, and a post-store DVE memset observes store completion, so
cross-engine waits stay single and the tail drains' DMA waits are
redundant (_legalize_waits drops provably-implied waits via a
happens-before closure).
"""

import ml_dtypes
import numpy as np

import bass_rust
import concourse.bass as bass
import concourse.mybir as mybir
import concourse.tile as tile
from concourse import library_config
from concourse.bass_utils import run_bass_kernel_spmd
from concourse.library_overlay import lower_extended_insts

B, H, W, C, P = 16, 128, 128, 256, 8192
NCORES = 8
BPC = B // NCORES        # batches per core
GCHUNK = 1024            # points per dma_gather call (= descriptors)
CPB = P // GCHUNK        # gather chunks per batch (8)
SLOTS = GCHUNK // 128    # 128-point slots per chunk (8)
KPB = P // 128           # slots per batch (64)
NI16 = P // 16           # id columns in 16-partition layout (512)
NWIN = (H - 1) * W       # row-pair windows per batch image (16256)

_f32 = mybir.dt.float32
_i16 = mybir.dt.int16
_bf16 = mybir.dt.bfloat16
_mul = mybir.AluOpType.mult
_add = mybir.AluOpType.add
_sub = mybir.AluOpType.subtract
_Copy = mybir.ActivationFunctionType.Copy


def build_nc() -> bass.Bass:
    nc = bass.Bass("TRN2", dynamic_dma_scratch_size=32768)
    y = nc.dram_tensor("y", [BPC * NWIN, 2 * C], _bf16, kind="ExternalInput")
    # idxw[lb*128+p, 2t+c] = idx[lb, t*128+p, c]   (weights layout)
    idxw = nc.dram_tensor("idxw", [BPC * 128, 2 * KPB], _f32,
                          kind="ExternalInput")
    # idxi[lb*128+q, 2s+c] = idx[lb, s*16+(q%16), c]  (id layout, x8 repl)
    idxi = nc.dram_tensor("idxi", [BPC * 128, 2 * NI16], _f32,
                          kind="ExternalInput")
    out = nc.dram_tensor("out", [BPC * P, C], _f32, kind="ExternalOutput")

    # Overlapping-window view: window r covers y rows r and r+1
    # (1024 floats = the 2x2 patch [TL | BL | TR | BR]).
    src_win = bass_rust.AP(y[:, :].tensor, 0,
                           [[2 * C, BPC * NWIN - 1], [1, 4 * C]])

    with tile.TileContext(nc) as tc:
        with (
            tc.tile_pool(name="prep", bufs=2) as pp,
            tc.tile_pool(name="persist", bufs=1) as ps,
            tc.tile_pool(name="gp", bufs=4) as gp,
            tc.tile_pool(name="tp", bufs=2) as tp,
            tc.tile_pool(name="vp", bufs=2) as vp,
            tc.tile_pool(name="up", bufs=6) as up,
            tc.tile_pool(name="op", bufs=4) as op,
        ):
            nc.gpsimd.load_library(library_config.mlp)

            ids16 = {}
            wts = {}
            wtlf32 = {}
            wblf32 = {}

            def prep_ids(lb):
                # --- window ids, idx-stream layout (x8 replicated by host)
                rawI = pp.tile([128, 2 * NI16], _f32, tag="rawI")
                nc.sync.dma_start(rawI[:], idxi[lb * 128:(lb + 1) * 128, :])
                rndI = pp.tile([128, 2 * NI16], _f32, tag="rndI")
                nc.vector.tensor_scalar(
                    rndI[:], rawI[:], 8388608.0, 8388608.0, _add, _sub
                )
                gtI = pp.tile([128, 2 * NI16], _f32, tag="gtI")
                nc.vector.tensor_tensor(gtI[:], rndI[:], rawI[:],
                                        mybir.AluOpType.is_gt)
                flrI = pp.tile([128, 2 * NI16], _f32, tag="flrI")
                nc.vector.tensor_tensor(flrI[:], rndI[:], gtI[:], _sub)
                topf = pp.tile([128, NI16], _f32, tag="topf")
                nc.vector.scalar_tensor_tensor(
                    topf[:], flrI[:, 0::2], float(W), flrI[:, 1::2], _mul, _add
                )
                ids = ps.tile([128, NI16], _i16, tag=f"ids{lb}")
                nc.vector.tensor_scalar(
                    ids[:, :], topf[:], float(lb * NWIN), None, _add
                )
                ids16[lb] = ids

            def prep_weights(lb):
                # --- corner weights (bf16), gather layout: (p,t) = t*128+p
                rawW = pp.tile([128, 2 * KPB], _f32, tag="rawW")
                nc.sync.dma_start(rawW[:], idxw[lb * 128:(lb + 1) * 128, :])
                rndW = pp.tile([128, 2 * KPB], _f32, tag="rndW")
                nc.vector.tensor_scalar(
                    rndW[:], rawW[:], 8388608.0, 8388608.0, _add, _sub
                )
                gtW = pp.tile([128, 2 * KPB], _f32, tag="gtW")
                nc.vector.tensor_tensor(gtW[:], rndW[:], rawW[:],
                                        mybir.AluOpType.is_gt)
                flrW = pp.tile([128, 2 * KPB], _f32, tag="flrW")
                nc.vector.tensor_tensor(flrW[:], rndW[:], gtW[:], _sub)
                mu = pp.tile([128, 2 * KPB], _f32, tag="mu")
                nc.vector.tensor_tensor(mu[:], rawW[:], flrW[:], _sub)
                mx = mu[:, 0::2]       # frac along h
                my = mu[:, 1::2]       # frac along w
                # corner weights: TL=(hf,wf) TR=(hf,wc) BL=(hc,wf) BR=(hc,wc)
                wbrf = pp.tile([128, KPB], _f32, tag="wbrf")
                nc.vector.tensor_tensor(wbrf[:], mx, my, _mul)
                wblf = pp.tile([128, KPB], _f32, tag="wblf")
                nc.vector.tensor_tensor(wblf[:], mx, wbrf[:], _sub)
                wtrf = pp.tile([128, KPB], _f32, tag="wtrf")
                nc.vector.tensor_tensor(wtrf[:], my, wbrf[:], _sub)
                sxy = pp.tile([128, KPB], _f32, tag="sxy")
                nc.vector.tensor_tensor(sxy[:], mx, my, _add)
                ap1 = pp.tile([128, KPB], _f32, tag="ap1")
                nc.vector.tensor_scalar(ap1[:], wbrf[:], 1.0, None, _add)
                wtlf = pp.tile([128, KPB], _f32, tag="wtlf")
                nc.vector.tensor_tensor(wtlf[:], ap1[:], sxy[:], _sub)
                ws = []
                for nm, wf in (("wtl", wtlf), ("wtr", wtrf),
                               ("wbl", wblf), ("wbr", wbrf)):
                    w16 = ps.tile([128, KPB], _bf16, tag=f"{nm}{lb}")
                    nc.scalar.activation(w16[:], wf[:], _Copy)
                    ws.append(w16)
                wts[lb] = tuple(ws)
                for nm, wf in (("wtl", wtlf), ("wbl", wblf)):
                    wp32 = ps.tile([128, KPB], _f32, tag=f"{nm}f32_{lb}")
                    nc.scalar.activation(wp32[:], wf[:], _Copy)
                    (wtlf32 if nm == "wtl" else wblf32)[lb] = wp32


            prep_ids(0)
            prep_ids(1)
            prep_weights(0)
            prep_weights(1)

            # --- gather + interpolate + store
            pending = []          # och tiles whose store sem is unobserved
            for lb in range(BPC):
                ids = ids16[lb]
                wtl, wtr, wbl, wbr = wts[lb]
                for k in range(CPB):
                    i0 = k * (GCHUNK // 16)
                    i1 = (k + 1) * (GCHUNK // 16)
                    g = gp.tile([128, SLOTS, 4 * C], _bf16, tag="g")
                    nc.gpsimd.dma_gather(
                        g[:], src_win, ids[:, i0:i1],
                        GCHUNK, GCHUNK, 4 * C, elem_step=2 * C,
                    )
                    # Deferred store observation: just before this chunk
                    # rewrites the och buffer (2 chunks after its store was
                    # issued, long since drained), a memset observes the
                    # store's completion sem on DVE -- off the critical
                    # path, unlike observing right after the store issue.
                    if len(pending) >= 4:
                        nc.vector.memset(pending.pop(0)[:, :1], 0.0)
                    och = op.tile([128, SLOTS * C], _f32, tag="och")
                    # Observe the gather's DMA sem once on each consumer
                    # engine, writing into disjoint columns of the output
                    # chunk so the touches carry no other dependency.
                    nc.vector.tensor_copy(och[:, 1:2], g[:, 0, 0:1])
                    tcha = op.tile([128, 1], _f32, tag="tcha")
                    nc.scalar.activation(tcha[:], g[:, 0, 1:2], _Copy)
                    vg1 = vp.tile([128, SLOTS, C], _bf16, tag="vg1")
                    vg2 = vp.tile([128, SLOTS, C], _bf16, tag="vg2")
                    for j in range(SLOTS):
                        t = k * SLOTS + j
                        # patch blocks: [TL | BL | TR | BR]
                        # out = wtl*TL + wbl*BL + wtr*TR + wbr*BR
                        u1 = up.tile([128, C], _bf16, tag="u1")
                        nc.scalar.activation(
                            u1[:], g[:, j, 0:C], _Copy,
                            bias=0.0, scale=wtlf32[lb][:, t:t + 1],
                        )
                        u2 = up.tile([128, C], _bf16, tag="u2")
                        nc.scalar.activation(
                            u2[:], g[:, j, C:2 * C], _Copy,
                            bias=0.0, scale=wblf32[lb][:, t:t + 1],
                        )
                        nc.vector.scalar_tensor_tensor(
                            vg1[:, j, :], g[:, j, 2 * C:3 * C], wtr[:, t:t + 1],
                            u1[:], _mul, _add,
                        )
                        nc.vector.scalar_tensor_tensor(
                            vg2[:, j, :], g[:, j, 3 * C:4 * C], wbr[:, t:t + 1],
                            u2[:], _mul, _add,
                        )
                    nc.vector.tensor_tensor(
                        och[:].rearrange("p (a b) -> p a b", a=SLOTS),
                        vg1[:], vg2[:], _add)
                    # dst[p, (j c)] = out[lb*P + (k*SLOTS+j)*128 + p, c]
                    nc.sync.dma_start(
                        bass_rust.AP(
                            out[:, :].tensor,
                            (lb * P + k * SLOTS * 128) * C,
                            [[C, 128], [128 * C, SLOTS], [1, C]],
                        ),
                        och[:],
                    )
                    pending.append(och)
            # tail: observe the final stores so the drain's DMA waits are
            # implied by DVE retirement
            for och in pending:
                nc.vector.memset(och[:, :1], 0.0)

    lower_extended_insts(nc)
    _legalize_waits(nc)
    return nc


def _legalize_waits(nc: bass.Bass) -> None:
    """Walrus codegen allows a single sync-wait per instruction.  Tile
    emits per-proc minimal waits but is not transitively minimal.  This
    pass computes a sound happens-before closure (vector clocks over
    semaphore events, walking the scheduled stream in order) and drops
    every wait already implied by the others; it asserts exactly one
    wait survives on every instruction that had several.

    Soundness notes: a proc executes its stream in order, and a wait
    stalls the proc's dispatch, so instruction i inherits all guarantees
    that held when the previous same-proc instruction dispatched.  A
    semaphore reaching value v implies the waits of the instructions
    that produced updates 1..v held; DMA-completion sems additionally
    imply the issuing instruction's engine-sem updates (completion
    happens after retirement), not vice versa."""

    def merge(a, b):
        for kk, vv in b.items():
            if a.get(kk, 0) < vv:
                a[kk] = vv

    cur: dict = {}        # proc -> VC (dict sem -> guaranteed value)
    events: dict = {}     # sem -> list of (cum_value, prefix-merged VC)
    cum: dict = {}        # sem -> cumulative update count
    # this kernel is a linear instruction stream; blocks execute in order
    for bb in nc.m.functions[0].blocks:

        def closure(s, v):
            evs = events.get(s)
            if not evs:
                return None
            for cv, vc in evs:           # events are few per sem; linear scan
                if cv >= v:
                    return vc
            return None

        for ins in bb.instructions:
            si = ins.sync_info
            eng = ins.engine
            begin = dict(cur.get(eng, {}))
            if si is not None:
                waits = list(si.on_wait)
                if len(waits) > 1:
                    # find one wait whose closure (with program-order
                    # guarantees) implies all the others
                    chosen = None
                    waits.sort(key=lambda w: w.ant_name.startswith("DMA"))
                    for w in waits:
                        trial = dict(begin)
                        c = closure(w.ant_name, w.wait_value)
                        if c is not None:
                            merge(trial, c)
                        if trial.get(w.ant_name, 0) < w.wait_value:
                            trial[w.ant_name] = w.wait_value
                        if all(trial.get(o.ant_name, 0) >= o.wait_value
                               for o in waits if o is not w):
                            chosen = w
                            begin = trial
                            break
                    assert chosen is not None, (
                        ins.name, type(ins).__name__,
                        [(w.ant_name, w.wait_value) for w in si.on_wait],
                    )
                    si.on_wait = [chosen]
                elif waits:
                    w = waits[0]
                    c = closure(w.ant_name, w.wait_value)
                    if c is not None:
                        merge(begin, c)
                    if begin.get(w.ant_name, 0) < w.wait_value:
                        begin[w.ant_name] = w.wait_value
                # register update events
                ups = list(si.on_update)
                retire = dict(begin)
                for u in ups:             # engine sems retire first
                    if not u.ant_name.startswith("DMA"):
                        cum[u.ant_name] = cum.get(u.ant_name, 0) + u.update_value
                        retire[u.ant_name] = cum[u.ant_name]
                for u in ups:
                    s = u.ant_name
                    if s.startswith("DMA"):
                        cum[s] = cum.get(s, 0) + u.update_value
                    vc = dict(retire)
                    vc[s] = cum[s]
                    prev = events.setdefault(s, [])
                    if prev:
                        base = dict(prev[-1][1])
                        merge(base, vc)
                        vc = base
                    prev.append((cum[s], vc))
            cur[eng] = begin


_NC = None


def _get_nc() -> bass.Bass:
    global _NC
    if _NC is None:
        _NC = build_nc()
    return _NC


def _in_maps(in_tensor: np.ndarray, indices: np.ndarray):
    maps = []
    for i in range(NCORES):
        xb = np.ascontiguousarray(
            in_tensor[i * BPC:(i + 1) * BPC], dtype=np.float32
        )  # [BPC, H, W, C]
        # row-pair windows: y[lb, h, w] = [x[lb,h,w,:], x[lb,h+1,w,:]]
        yb = np.concatenate([xb[:, :-1], xb[:, 1:]], axis=-1)
        yb = yb.astype(ml_dtypes.bfloat16)
        idx = np.ascontiguousarray(
            indices[i * BPC:(i + 1) * BPC], dtype=np.float32
        )  # [BPC, P, 2]
        idxw = idx.reshape(BPC, KPB, 128, 2).transpose(0, 2, 1, 3)
        base = idx.reshape(BPC, NI16, 16, 2).transpose(0, 2, 1, 3)
        idxi = np.tile(base.reshape(BPC, 16, 2 * NI16), (1, 8, 1))
        maps.append(
            {
                "y": yb.reshape(BPC * NWIN, 2 * C),
                "idxw": np.ascontiguousarray(
                    idxw.reshape(BPC * 128, 2 * KPB)
                ),
                "idxi": np.ascontiguousarray(
                    idxi.reshape(BPC * 128, 2 * NI16)
                ),
            }
        )
    return maps


def kernel(in_tensor: np.ndarray, indices: np.ndarray) -> np.ndarray:
    nc = _get_nc()
    res = run_bass_kernel_spmd(
        nc, _in_maps(in_tensor, indices), core_ids=list(range(NCORES))
    )
    return np.concatenate(
        [res.results[i]["out"].reshape(BPC, P, C) for i in range(NCORES)], axis=0
    )
